# revision 40
# baseline (speedup 1.0000x reference)
"""AdaptiveTemporalEncoding distributed Bass kernel for 8 TRN2 NeuronCores.

Final design (no inter-core collective, minimal PE batch-replication):
  Host supplies per-(batch-group, drug) counts cnt[g*15+d, l] (counts <= 8,
  exact in fp8) REPLICATED to every core, so each core computes the
  full-batch sum locally via the projection matmuls and the AllGather
  disappears. Host also pre-expands bolus*mask into hm[lp, (lt,b,d)] bf16
  (a rank-1 input product, like the baseline's onehot expansion).

Per-core pipeline:
  1. projection: S[(c|i), l'] = sum_{g,d} dfm3[(g,d), i] * cnt[(g,d), l]
     (dfm/64 split into 3 exact bf16 terms; fp8 counts; f32 psum; L folded
     into 128 partitions = 2 halves x 64 freqs, LH=2048 free; all 4 banks
     issued up-front with prj bufs=4 so PE never waits on the scan chain)
  2. exact hi/lo split prefix-scan over time: all 8 scans are INDEPENDENT
     (zero-init) on Vector; inter-bank carries folded in via per-bank
     prefix columns pcol (exact dyadic hi totals) in the csum STT
  3. carry for the c=1 partition half via shift64 matmul + reduce
  4. phases per 512-chunk: a = (csum+carry)*A1 + A0 (host tables A0=t*f,
     A1=t*f/(t+1)); k = round(a/2pi) on ACT; s2 via fused
     cody_waite_cascade; cos argument via add_range_wrap(s2 + pi/2);
     Sin on ACT
  5. combined sin|cos tile svcv_h [128 = 64 sin + 64 cos, l] per half
     (two small SBUF->SBUF DMAs move the cos halves into place)
  6. enc-only formation: ONE 128-col interleave matmul per 128-l tile
     (no batch replication on PE); ACT drains psum -> enc_sb
  7. batch replication + bolus*mask + bf16 convert fused into ONE Vector
     tensor_tensor per l-tile: stage = hm_tile + broadcast(enc_tile)
     (stride-0 batch-broadcast AP); per-tile store DMA [lp, b, d]
"""
import sys
sys.path.insert(0, '/opt/trn_rl_repo')
import numpy as np
import ml_dtypes

from concourse import bass, bacc, mybir, tile
from concourse.bass_utils import run_bass_kernel_spmd

dt = mybir.dt
F32, BF16, I32 = dt.float32, dt.bfloat16, dt.int32
F8 = dt.float8e4
Alu = mybir.AluOpType
Act = mybir.ActivationFunctionType

B, L, D, NDT = 64, 4096, 128, 15
D2 = D // 2                      # 64
NCORES = 8
BS = B // NCORES                 # 8 batches per core
NG = 8                           # global batch groups of 8
KP = NG * NDT                    # 120 rows, row g*15 + d
LH = L // 2                      # 2048 (free width of the (c|l') layout)
NBANK = 4                        # LH split into 4 banks of 512 for proj/scan
NCHUNK = 4                       # phase pipeline chunks of 512
CHW = LH // NCHUNK               # 512
NIDX = 4096                      # padded masked-position count (mean ~3277)
NIW = NIDX // 128                # 32 index columns

TWO_PI = float(2.0 * np.pi)
C1 = 6.28125                                     # k*C1 exact for our k range
C2 = float(np.float32(TWO_PI - C1))              # residue
INV2PI = float(np.float32(1.0 / TWO_PI))
PI_F = float(np.float32(np.pi))
PI_HALF = float(np.float32(np.pi / 2))
S14 = float(2.0 ** 14)
SM14 = float(2.0 ** -14)

CORE_IDS = list(range(NCORES))
OUT_BF16 = True
ODT_NP = ml_dtypes.bfloat16 if OUT_BF16 else np.float32


def _build(repeat=1):
    nc = bacc.Bacc("TRN2", target_bir_lowering=False, debug=False,
                   num_devices=NCORES)
    nc.all_engine_barrier()

    # ---- inputs (cnt/consts identical across cores; indices per-core) ----
    cnt_in = nc.declare_dram_parameter("cnt", [KP, L], F8, isOutput=False)
    cstbf_in = nc.declare_dram_parameter("cstbf", [128, 1024], BF16, isOutput=False)
    a0a1_in = nc.declare_dram_parameter("a0a1", [128, 3 * LH], F32, isOutput=False)
    shift64_in = nc.declare_dram_parameter("shift64", [64, 128], F32, isOutput=False)
    hm_in = nc.declare_dram_parameter("hm", [128, 32 * 1024], BF16, isOutput=False)

    ODT = BF16 if OUT_BF16 else F32
    out_ext = nc.declare_dram_parameter("out", [BS, L, D], ODT, isOutput=True)
    # out viewed per 128-l tile: [lt][lp, b, d]
    ov = out_ext.ap().rearrange("b (lt lp) d -> lt lp b d", lp=128)

    with tile.TileContext(nc) as tc:
        with (
            tc.tile_pool(name="cst", bufs=1) as cst,
            tc.tile_pool(name="sb", bufs=1) as sb,
            tc.tile_pool(name="bk", bufs=1) as bk,
            tc.tile_pool(name="ph", bufs=3) as ph,
            tc.tile_pool(name="sv", bufs=4) as svp,
            tc.tile_pool(name="prj", bufs=4, space="PSUM") as prj,
            tc.tile_pool(name="acc", bufs=3, space="PSUM") as accp,
            tc.tile_pool(name="stg", bufs=4) as stg,
            tc.tile_pool(name="dram", bufs=1, space="DRAM") as dram,
        ):
            # ---------- load inputs ----------
            cnt = cst.tile([KP, L], F8, tag="cnt")
            for q in range(4):
                nc.sync.dma_start(cnt[:, q * 1024:(q + 1) * 1024],
                                  cnt_in[:, q * 1024:(q + 1) * 1024])
            cstbf = cst.tile([128, 1024], BF16, tag="cstbf")
            nc.sync.dma_start(cstbf[:], cstbf_in[:])
            a0a1 = cst.tile([128, 3 * LH], F32, tag="a0a1")
            for q in range(3):
                nc.sync.dma_start(a0a1[:, q * LH:(q + 1) * LH],
                                  a0a1_in[:, q * LH:(q + 1) * LH])
            shift64 = cst.tile([64, 128], F32, tag="shift64")
            nc.sync.dma_start(shift64[:], shift64_in[:])
            hm = cst.tile([128, 32 * 1024], BF16, tag="hm")
            for q in range(4):
                nc.scalar.dma_start(hm[:, q * 8192:(q + 1) * 8192],
                                    hm_in[:, q * 8192:(q + 1) * 8192])

            dfm3 = [cstbf[0:KP, i * 128:(i + 1) * 128] for i in range(6)]
            icomb = [cstbf[:, 768 + h * 128: 768 + (h + 1) * 128] for h in range(2)]
            A0 = a0a1[:, 0:LH]
            A1 = a0a1[:, LH:2 * LH]
            TP1 = a0a1[:, 2 * LH:3 * LH]

            zerosF = sb.tile([128, 512], F32, tag="zF")
            nc.gpsimd.memset(zerosF[:], 0.0)
            ch_t = sb.tile([128, LH], F32, tag="chi")
            cl_t = sb.tile([128, LH], F32, tag="clo")
            csum_t = sb.tile([128, LH], F32, tag="csum")
            tsum_t = sb.tile([128, 8], F32, tag="tsum")
            carry_col = sb.tile([128, 1], F32, tag="carry")
            enc_sb = sb.tile([128, L], ODT, tag="enc")

            for _rep in range(repeat):
                # ---------- projection + hi/lo split (all banks up front) ----
                shis, slos = [], []
                for bank in range(NBANK):
                    pj = prj.tile([128, 512], F32, tag="prj")
                    for c in range(2):
                        src = cnt[:, c * LH + bank * 512: c * LH + (bank + 1) * 512]
                        for i in range(3):
                            nc.tensor.matmul(pj[:], dfm3[3 * c + i], src,
                                             start=(c == 0 and i == 0),
                                             stop=(c == 1 and i == 2))
                    k1 = bk.tile([128, 512], I32, tag=f"k1{bank}")
                    shi = bk.tile([128, 512], F32, tag=f"shi{bank}")
                    slo = bk.tile([128, 512], F32, tag=f"slo{bank}")
                    nc.scalar.activation(k1[:], pj[:], Act.Copy, scale=S14)
                    nc.scalar.activation(shi[:], k1[:], Act.Copy, scale=SM14,
                                         accum_out=tsum_t[:, bank:bank + 1])
                    nc.vector.scalar_tensor_tensor(
                        slo[:], shi[:], -1.0, pj[:], Alu.mult, Alu.add,
                        accum_out=tsum_t[:, 4 + bank:5 + bank])
                    shis.append(shi)
                    slos.append(slo)
                # per-bank prefix columns pcol[b] = sum_{k<b}(hi_tot + lo_tot)
                # (exact: hi totals are dyadic, lo totals tiny) so all 8 scans
                # run independently (zero-init): hi on Vector, lo on GpSimd.
                pcol = sb.tile([128, 4], F32, tag="pcol")
                for b in range(1, 4):
                    u = b - 1
                    nc.vector.tensor_tensor(pcol[:, b - 1:b], tsum_t[:, u:u + 1],
                                            tsum_t[:, 4 + u:5 + u], Alu.add)
                    if b > 1:
                        nc.vector.tensor_tensor(pcol[:, b - 1:b],
                                                pcol[:, b - 1:b],
                                                pcol[:, b - 2:b - 1], Alu.add)
                for bank in range(NBANK):
                    B0 = slice(bank * 512, (bank + 1) * 512)
                    nc.vector.tensor_tensor_scan(ch_t[:, B0], shis[bank][:],
                                                 zerosF[:], 0.0, Alu.add, Alu.add)
                    nc.vector.tensor_tensor_scan(cl_t[:, B0], slos[bank][:],
                                                 zerosF[:], 0.0, Alu.add, Alu.add)
                    if bank == 0:
                        nc.vector.tensor_tensor(csum_t[:, B0], ch_t[:, B0],
                                                cl_t[:, B0], Alu.add)
                    else:
                        nc.vector.scalar_tensor_tensor(
                            csum_t[:, B0], ch_t[:, B0],
                            pcol[:, bank - 1:bank], cl_t[:, B0],
                            Alu.add, Alu.add)

                # carry for the c=1 partition half: total of c=0 bank sums,
                # shifted to partitions 64..127 (zero on 0..63)
                p_c = prj.tile([128, 512], F32, tag="prj")
                nc.tensor.matmul(p_c[:, 0:8], shift64[:], tsum_t[0:64, :],
                                 start=True, stop=True)
                nc.vector.tensor_reduce(carry_col[:], p_c[:, 0:8],
                                        mybir.AxisListType.X, Alu.add)

                # ---------- software-pipelined phases + output ----------
                def emit_phases(chk):
                    F = slice(chk * CHW, (chk + 1) * CHW)
                    tmp = ph.tile([128, CHW], F32, tag="tmp")
                    a_t = ph.tile([128, CHW], F32, tag="a")
                    k_t = ph.tile([128, CHW], I32, tag="k")
                    s2 = ph.tile([128, CHW], F32, tag="s2")
                    w2 = ph.tile([128, CHW], F32, tag="w2")
                    # a = (csum + carry) * A1 + A0   (A0=t*f, A1=t*f/(t+1))
                    nc.vector.scalar_tensor_tensor(tmp[:], csum_t[:, F], carry_col[:],
                                                   A1[:, F], Alu.add, Alu.mult)
                    nc.vector.tensor_tensor(a_t[:], tmp[:], A0[:, F], Alu.add)
                    nc.scalar.activation(k_t[:], a_t[:], Act.Copy, scale=INV2PI)
                    nc.vector.cody_waite_cascade(s2[:], a_t[:], k_t[:], C1, C2, 0.0)
                    nc.vector.add_range_wrap(w2[:], s2[:], PI_HALF, PI_F, TWO_PI)
                    sv0 = svp.tile([128, CHW], BF16, tag="sv0")
                    sv1 = svp.tile([128, CHW], BF16, tag="sv1")
                    cosT = svp.tile([128, CHW], BF16, tag="cosT")
                    nc.scalar.activation(sv0[0:64, :], s2[0:64, :], Act.Sin)
                    nc.scalar.activation(sv1[64:128, :], s2[64:128, :], Act.Sin)
                    nc.scalar.activation(cosT[:], w2[:], Act.Sin)
                    nc.scalar.dma_start(sv0[64:128, :], cosT[0:64, :])
                    nc.scalar.dma_start(sv1[0:64, :], cosT[64:128, :])
                    return sv0, sv1

                def emit_output(chk, sv0, sv1):
                    for h in range(2):
                        svh = sv0 if h == 0 else sv1
                        lt0 = h * 16 + chk * 4
                        po = accp.tile([128, 512], F32, tag="acc")
                        for j in range(4):
                            nc.tensor.matmul(po[:, j * 128:(j + 1) * 128],
                                             svh[:, j * 128:(j + 1) * 128],
                                             icomb[h], start=True, stop=True)
                        E = slice(lt0 * 128, lt0 * 128 + 512)
                        nc.scalar.copy(enc_sb[:, E], po[:])
                        for j in range(4):
                            lt = lt0 + j
                            enc_b = enc_sb[:, lt * 128:(lt + 1) * 128].rearrange(
                                "p (x d) -> p x d", x=1).to_broadcast([128, BS, D])
                            hmv = hm[:, lt * 1024:(lt + 1) * 1024]
                            stage = stg.tile([128, 1024], ODT, tag="st")
                            nc.vector.tensor_tensor(stage[:], hmv, enc_b, Alu.add)
                            nc.sync.dma_start(
                                ov[lt],
                                stage[:].rearrange("p (b d) -> p b d", b=BS))

                for chk in range(NCHUNK):
                    sv0, sv1 = emit_phases(chk)
                    emit_output(chk, sv0, sv1)

    nc.compile()
    return nc


_CACHED = {}


def _get_nc(repeat=1):
    if repeat not in _CACHED:
        _CACHED[repeat] = _build(repeat)
    return _CACHED[repeat]


def _host_inputs(bolus_mask, dominant_drugs, base_frequencies,
                 drug_freq_modulation, bolus_embedding):
    mask = np.asarray(bolus_mask).astype(bool)                # [B, L]
    drugs = np.asarray(dominant_drugs).astype(np.int64)       # [B, L]
    basef = np.asarray(base_frequencies).astype(np.float64)   # [D2]
    dfm = np.asarray(drug_freq_modulation).astype(np.float32) # [NDT, D2]
    bolus = np.asarray(bolus_embedding).astype(np.float32).reshape(D)

    # per-(group, drug) counts over the FULL batch; exact in fp8 (<= 8)
    dall = drugs.reshape(NG, BS, L)
    cnt = np.zeros((KP, L), ml_dtypes.float8_e4m3)
    for d in range(NDT):
        c = (dall == d).sum(axis=1)                           # [NG, L]
        cnt[np.arange(NG) * NDT + d] = c.astype(ml_dtypes.float8_e4m3)

    # dfm replicated over groups (row g*NDT+d) and split into 3 exact bf16
    # terms of dfm/64 (so the matmul computes the batch MEAN numerator)
    dfm_rep = np.tile(dfm, (NG, 1)).astype(np.float64) / 64.0  # [120, 64]
    d_hi = dfm_rep.astype(ml_dtypes.bfloat16)
    r1 = (dfm_rep - d_hi.astype(np.float64)).astype(np.float32)
    d_mid = r1.astype(ml_dtypes.bfloat16)
    r2 = (r1.astype(np.float64) - d_mid.astype(np.float64)).astype(np.float32)
    d_lo = r2.astype(ml_dtypes.bfloat16)

    cstbf = np.zeros((128, 1024), ml_dtypes.bfloat16)
    for c in range(2):
        for i, dpart in enumerate((d_hi, d_mid, d_lo)):
            blk = np.zeros((KP, 128), ml_dtypes.bfloat16)
            blk[:, 64 * c:64 * c + 64] = dpart
            cstbf[0:KP, (3 * c + i) * 128:(3 * c + i + 1) * 128] = blk
    # narrow interleave matrices [128, 128]:
    #   h=0 rows 0:64 sin->even, 64:128 cos->odd
    #   h=1 rows 0:64 cos->odd,  64:128 sin->even
    for h in range(2):
        ic = np.zeros((128, 128), ml_dtypes.bfloat16)
        if h == 0:
            ic[np.arange(64), 2 * np.arange(64)] = 1
            ic[64 + np.arange(64), 2 * np.arange(64) + 1] = 1
        else:
            ic[np.arange(64), 2 * np.arange(64) + 1] = 1
            ic[64 + np.arange(64), 2 * np.arange(64)] = 1
        cstbf[:, 768 + h * 128:768 + (h + 1) * 128] = ic

    shift64 = np.zeros((64, 128), np.float32)
    shift64[np.arange(64), np.arange(64) + 64] = 1.0

    # phase tables: t(p, col) = col + 2048*(p>=64); f(p) = basef[p%64]
    t_idx = ((np.arange(128)[:, None] // 64) * LH
             + np.arange(LH)[None, :]).astype(np.float64)
    f_col = np.tile(basef, 2).reshape(128, 1)
    A0 = (f_col * t_idx)
    A1 = A0 / (t_idx + 1.0)
    TP1 = t_idx + 1.0
    a0a1 = np.concatenate([A0, A1, TP1], axis=1).astype(np.float32)



    in_maps = []
    for r in range(NCORES):
        msh = mask[r * BS:(r + 1) * BS]                       # [8, L] bool
        # hm[lp, lt*1024 + b*128 + d] = mask[b, lt*128+lp] * bolus[d]
        m3 = msh.T.reshape(32, 128, BS).transpose(1, 0, 2)    # [lp, lt, b]
        hmc = (m3[:, :, :, None].astype(np.float32)
               * bolus[None, None, None, :]).astype(ml_dtypes.bfloat16)
        in_maps.append(dict(
            cnt=cnt, cstbf=cstbf, a0a1=a0a1, shift64=shift64,
            hm=np.ascontiguousarray(hmc.reshape(128, 32 * 1024)),
        ))
    return in_maps


_RUNNER = None


def _make_runner():
    """Build the jitted PJRT executable once; reuse across kernel() calls."""
    import jax
    from jax.experimental.shard_map import shard_map
    from jax.sharding import Mesh, PartitionSpec, NamedSharding
    from concourse import bass2jax

    nc = _get_nc(1)
    bass2jax.install_neuronx_cc_hook()
    partition_name = nc.partition_id_tensor.name if nc.partition_id_tensor else None

    in_names, out_names, out_avals = [], [], []
    for alloc in nc.m.functions[0].allocations:
        if not isinstance(alloc, mybir.MemoryLocationSet):
            continue
        name = alloc.memorylocations[0].name
        if alloc.kind == "ExternalInput":
            if name != partition_name:
                in_names.append(name)
        elif alloc.kind == "ExternalOutput":
            out_names.append(name)
            out_avals.append(jax.core.ShapedArray(
                tuple(alloc.tensor_shape), mybir.dt.np(alloc.dtype)))
    n_params = len(in_names)
    all_in_names = list(in_names) + list(out_names)
    if partition_name is not None:
        all_in_names.append(partition_name)

    def _body(*args):
        operands = list(args)
        if partition_name is not None:
            operands.append(bass2jax.partition_id_tensor())
        outs = bass2jax._bass_exec_p.bind(
            *operands,
            out_avals=tuple(out_avals),
            in_names=tuple(all_in_names),
            out_names=tuple(out_names),
            lowering_input_output_aliases=(),
            sim_require_finite=True,
            sim_require_nnan=True,
            nc=nc,
        )
        return tuple(outs)

    devices = jax.devices()[:NCORES]
    mesh = Mesh(np.asarray(devices), ("core",))
    n_args = n_params + len(out_names)
    sharded = jax.jit(
        shard_map(_body, mesh=mesh,
                  in_specs=(PartitionSpec("core"),) * n_args,
                  out_specs=(PartitionSpec("core"),) * len(out_names),
                  check_rep=False),
        keep_unused=True,
    )
    sh = NamedSharding(mesh, PartitionSpec("core"))
    zero_dev = [jax.device_put(
        np.zeros((NCORES * a.shape[0], *a.shape[1:]), a.dtype), sh)
        for a in out_avals]

    def run(in_maps):
        per_core = [[np.asarray(m[name]) for name in in_names] for m in in_maps]
        concat_in = [np.concatenate([per_core[c][i] for c in range(NCORES)], axis=0)
                     for i in range(n_params)]
        dev_in = [jax.device_put(a, sh) for a in concat_in]
        out_arrs = sharded(*dev_in, *zero_dev)
        res = np.asarray(out_arrs[0])   # [NCORES*BS, L, D]
        return res

    return run


def kernel(seq_len, bolus_mask, dominant_drugs, base_frequencies,
           drug_freq_modulation, bolus_embedding):
    global _RUNNER
    assert int(seq_len) == L
    if _RUNNER is None:
        _RUNNER = _make_runner()
    in_maps = _host_inputs(bolus_mask, dominant_drugs, base_frequencies,
                           drug_freq_modulation, bolus_embedding)
    out = _RUNNER(in_maps)
    return np.ascontiguousarray(out.astype(np.float32))
